# revision 1
# baseline (speedup 1.0000x reference)
"""Trainium2 Bass kernel for AntecedentShareTriMF.

Computation (see reference):
  mf[b,d,m] = relu(min((x-c)/ld2 + 1, -(x-c)/rd2 + 1))        [B, D, M]
  frs[b,r]  = prod_d mf[b, d, rule_idx[r, d]]                  [B, R]
  out       = frs / (sum_r frs + eps)

With the cartesian-product rule table (M=2, D=10, R=2^10) the frs row
factors into an outer product of two 32-wide half-products:
  A[b, ra] = prod_{d<5}  mf[b, d, bit_d(ra)]
  B[b, rb] = prod_{d>=5} mf[b, d, bit_d(rb)]
  frs[b, ra*32+rb] = A[b,ra] * B[b,rb],   sum_r frs = (sum A)(sum B)
so the per-row work is ~1 multiply per output element instead of ~20.

Distribution: pure data parallel, batch sharded 8 ways (2048 rows/core),
tiny MF coefficients replicated. No collectives needed.
"""

import sys

for _p in ("/opt/trn_rl_repo", "/opt/pypackages"):
    if _p not in sys.path:
        sys.path.insert(0, _p)

import numpy as np

IN_DIM = 10
N_MF = 2
BATCH = 16384
N_RULE = 1024
N_CORES = 8
SHARD = BATCH // N_CORES          # 2048 rows per core
T = SHARD // 128                  # 16 rows per partition (block layout)
EPS = 1e-8
HALF = 32                         # 2^5 combinations per half

_prog_cache = {}


def _build_program():
    """Build + compile the single-core SPMD Bass program (once per process)."""
    if "nc" in _prog_cache:
        return _prog_cache["nc"]

    import concourse.bass as bass
    import concourse.bacc as bacc
    import concourse.mybir as mybir
    import concourse.tile as tile

    F32 = mybir.dt.float32
    AX = mybir.AxisListType
    OP = mybir.AluOpType

    nc = bacc.Bacc("TRN2", target_bir_lowering=False, debug=False,
                   num_devices=N_CORES)

    x_ext = nc.dram_tensor("X", [SHARD, IN_DIM], F32, kind="ExternalInput").ap()
    # coef layout per m (m-major): [-center, 1/ld2, -1/rd2] each [IN_DIM]
    coef_ext = nc.dram_tensor("coef", [128, 6 * IN_DIM], F32,
                              kind="ExternalInput").ap()
    out_ext = nc.dram_tensor("out", [SHARD, N_RULE], F32,
                             kind="ExternalOutput").ap()

    with tile.TileContext(nc) as tc:
        with (
            tc.tile_pool(name="const", bufs=1) as constp,
            tc.tile_pool(name="xin", bufs=1) as xinp,
            tc.tile_pool(name="mf", bufs=1) as mfp,
            tc.tile_pool(name="scratch", bufs=1) as scr,
            tc.tile_pool(name="halves", bufs=1) as halves,
            tc.tile_pool(name="outp", bufs=4) as outp,
        ):
            coef = constp.tile([128, 6 * IN_DIM], F32)
            nc.sync.dma_start(coef[:], coef_ext[:])

            # X in block layout: partition p holds rows p*T .. p*T+T-1
            xt = xinp.tile([128, T * IN_DIM], F32)
            nc.sync.dma_start(
                xt[:].rearrange("p (t d) -> p t d", d=IN_DIM),
                x_ext.rearrange("(p t) d -> p t d", t=T),
            )
            xt3 = xt[:].rearrange("p (t d) -> p t d", d=IN_DIM)

            # mf values, layout (t, d, m) with m innermost
            mfc = mfp.tile([128, T * IN_DIM * N_MF], F32)
            mfc4 = mfc[:].rearrange("p (t d m) -> p t d m", d=IN_DIM, m=N_MF)

            u = scr.tile([128, T * IN_DIM], F32, tag="u")
            t1 = scr.tile([128, T * IN_DIM], F32, tag="t1")
            t2 = scr.tile([128, T * IN_DIM], F32, tag="t2")
            u3 = u[:].rearrange("p (t d) -> p t d", d=IN_DIM)
            t13 = t1[:].rearrange("p (t d) -> p t d", d=IN_DIM)
            t23 = t2[:].rearrange("p (t d) -> p t d", d=IN_DIM)

            def cslice(i):  # i-th group of IN_DIM coefficients, bcast over t
                return (coef[:, i * IN_DIM:(i + 1) * IN_DIM]
                        .unsqueeze(1).to_broadcast([128, T, IN_DIM]))

            for m in range(N_MF):
                # u = x - center ; t1 = u/ld2 ; t2 = -u/rd2
                nc.vector.tensor_add(u3, xt3, cslice(3 * m + 0))
                nc.vector.tensor_mul(t13, u3, cslice(3 * m + 1))
                nc.vector.tensor_mul(t23, u3, cslice(3 * m + 2))
                nc.vector.tensor_tensor(t13, t13, t23, OP.min)
                # mf = max(min+1, 0), written interleaved (m innermost)
                nc.vector.tensor_scalar(mfc4[:, :, :, m], t13, 1.0, 0.0,
                                        OP.add, OP.max)

            # half-products by successive doubling
            A = halves.tile([128, T * HALF], F32, tag="A")
            B = halves.tile([128, T * HALF], F32, tag="B")

            def build_half(d0, out_tile):
                cur = mfc4[:, :, d0, :]                      # [128, T, 2]
                width = 2
                for k in range(1, 5):
                    nxt_width = width * 2
                    if k < 4:
                        nxt = scr.tile([128, T * nxt_width], F32,
                                       tag=f"dbl{k}")
                        nxt_v = nxt[:].rearrange("p (t j) -> p t j",
                                                 j=nxt_width)
                    else:
                        nxt_v = out_tile[:].rearrange("p (t j) -> p t j",
                                                      j=nxt_width)
                    nc.vector.tensor_mul(
                        nxt_v.rearrange("p t (j i) -> p t j i", i=2),
                        cur.unsqueeze(3).to_broadcast([128, T, width, 2]),
                        mfc4[:, :, d0 + k, :].unsqueeze(2)
                            .to_broadcast([128, T, width, 2]),
                    )
                    cur = nxt_v
                    width = nxt_width

            build_half(0, A)
            build_half(5, B)

            A3 = A[:].rearrange("p (t j) -> p t j", j=HALF)
            B3 = B[:].rearrange("p (t j) -> p t j", j=HALF)

            # row sums: sum_r frs = (sum A)(sum B); fold recip into A
            sa = scr.tile([128, T], F32, tag="sa")
            sb = scr.tile([128, T], F32, tag="sb")
            nc.vector.reduce_sum(sa[:].unsqueeze(2), A3, axis=AX.X)
            nc.vector.reduce_sum(sb[:].unsqueeze(2), B3, axis=AX.X)
            nc.vector.tensor_mul(sa[:], sa[:], sb[:])
            nc.vector.tensor_scalar_add(sa[:], sa[:], EPS)
            nc.vector.reciprocal(sb[:], sa[:])
            nc.vector.tensor_mul(
                A3, A3, sb[:].unsqueeze(2).to_broadcast([128, T, HALF]))

            out_r = out_ext.rearrange("(p t) r -> p t r", t=T)
            for t in range(T):
                o = outp.tile([128, N_RULE], F32)
                nc.vector.tensor_mul(
                    o[:].rearrange("p (a b) -> p a b", b=HALF),
                    A3[:, t, :].unsqueeze(2).to_broadcast([128, HALF, HALF]),
                    B3[:, t, :].unsqueeze(1).to_broadcast([128, HALF, HALF]),
                )
                nc.sync.dma_start(out_r[:, t, :], o[:])

    nc.compile()
    _prog_cache["nc"] = nc
    return nc


def _host_coefs(center, left_dist, right_dist):
    """[128, 60] replicated coefficient tile; layout per m:
    [-center, 1/ld2, -1/rd2], each IN_DIM wide."""
    c = np.asarray(center, np.float32)
    ld2 = np.asarray(left_dist, np.float32) ** 2 + np.float32(EPS)
    rd2 = np.asarray(right_dist, np.float32) ** 2 + np.float32(EPS)
    blocks = []
    for m in range(N_MF):
        blocks += [-c[:, m],
                   (1.0 / ld2[:, m].astype(np.float64)).astype(np.float32),
                   (-1.0 / rd2[:, m].astype(np.float64)).astype(np.float32)]
    row = np.concatenate(blocks).astype(np.float32)
    return np.ascontiguousarray(np.broadcast_to(row, (128, row.size)))


def _numpy_reference(X, center, left_dist, right_dist, rule_idx):
    """Safety-net path for non-cartesian rule tables (not the graded case)."""
    X = np.asarray(X, np.float32)
    center = np.asarray(center, np.float32)
    ld2 = np.asarray(left_dist, np.float32) ** 2 + np.float32(EPS)
    rd2 = np.asarray(right_dist, np.float32) ** 2 + np.float32(EPS)
    left = X[:, :, None] / ld2 + 1.0 - center / ld2
    right = -X[:, :, None] / rd2 + 1.0 + center / rd2
    mf = np.maximum(0.0, np.minimum(left, right)).astype(np.float32)
    frs = np.ones((X.shape[0], rule_idx.shape[0]), np.float32)
    for d in range(IN_DIM):
        frs = frs * mf[:, d, rule_idx[:, d]]
    return frs / (frs.sum(axis=1, keepdims=True) + np.float32(EPS))


def kernel(X, center, left_dist, right_dist, rule_idx):
    X = np.ascontiguousarray(np.asarray(X, np.float32))
    rule_idx = np.asarray(rule_idx, np.int32)
    assert X.shape == (BATCH, IN_DIM)

    # fast path requires the standard cartesian-product rule table
    # (itertools.product order: dim 0 is the most significant bit)
    weights = (2 ** np.arange(IN_DIM - 1, -1, -1)).astype(np.int64)
    codes = rule_idx.astype(np.int64) @ weights
    if (rule_idx.shape != (N_RULE, IN_DIM)
            or rule_idx.min() < 0 or rule_idx.max() >= N_MF
            or not np.array_equal(codes, np.arange(N_RULE))):
        return _numpy_reference(X, center, left_dist, right_dist, rule_idx)

    from concourse import bass_utils

    nc = _build_program()
    coef = _host_coefs(center, left_dist, right_dist)
    in_maps = [
        {"X": np.ascontiguousarray(X[c * SHARD:(c + 1) * SHARD]), "coef": coef}
        for c in range(N_CORES)
    ]
    res = bass_utils.run_bass_kernel_spmd(
        nc, in_maps, core_ids=list(range(N_CORES)))
    return np.concatenate([res.results[c]["out"] for c in range(N_CORES)],
                          axis=0)


# revision 3
# speedup vs baseline: 1.0404x; 1.0404x over previous
"""Trainium2 Bass kernel for AntecedentShareTriMF.

Computation (see reference):
  mf[b,d,m] = relu(min((x-c)/ld2 + 1, -(x-c)/rd2 + 1))        [B, D, M]
  frs[b,r]  = prod_d mf[b, d, rule_idx[r, d]]                  [B, R]
  out       = frs / (sum_r frs + eps)

With the cartesian-product rule table (M=2, D=10, R=2^10) each frs row
factors into an outer product of two 32-wide half-products over dims
0-4 / 5-9, and the row sum factors as prod_d (mf0[d] + mf1[d]), so the
per-row work is ~1 multiply per output element instead of ~20.

Distribution: pure data parallel, batch sharded 8 ways (2048 rows/core),
tiny MF coefficients replicated. No collectives needed.

Device schedule per core (memory-bound: 8 MB of output writes):
  - stacked-m MF evaluation (5 vector ops over [128, 320])
  - joint A/B-half successive doubling, new bit appended high
    (4 vector ops, halves stacked in one tensor)
  - row-sum via pairwise product tree + reciprocal
  - 16 outer-product combines [128,32x32] with the 1/rowsum scale fused
    via scalar_tensor_tensor; a few run on GpSimd to keep ahead of DMA
  - per-group 512 KB output DMAs alternating sync/scalar HWDGE rings
"""

import sys

for _p in ("/opt/trn_rl_repo", "/opt/pypackages"):
    if _p not in sys.path:
        sys.path.insert(0, _p)

import numpy as np

IN_DIM = 10
N_MF = 2
BATCH = 16384
N_RULE = 1024
N_CORES = 8
SHARD = BATCH // N_CORES          # 2048 rows per core
T = SHARD // 128                  # 16 rows per partition (block layout)
EPS = 1e-8
HALF = 32                         # 2^5 combinations per half
GP_GROUPS = ()                    # group ids combined on GpSimd

_prog_cache = {}


def _build_program():
    """Build + compile the single-core SPMD Bass program (once per process)."""
    if "nc" in _prog_cache:
        return _prog_cache["nc"]

    import concourse.bass as bass
    import concourse.bacc as bacc
    import concourse.mybir as mybir
    import concourse.tile as tile

    F32 = mybir.dt.float32
    OP = mybir.AluOpType

    nc = bacc.Bacc("TRN2", target_bir_lowering=False, debug=False,
                   num_devices=N_CORES)

    x_ext = nc.dram_tensor("X", [SHARD, IN_DIM], F32, kind="ExternalInput").ap()
    # coef rows: [-center | 1/ld2 | -1/rd2], each [IN_DIM*N_MF] (d,m)-interleaved
    coef_ext = nc.dram_tensor("coef", [128, 3 * IN_DIM * N_MF], F32,
                              kind="ExternalInput").ap()
    out_ext = nc.dram_tensor("out", [SHARD, N_RULE], F32,
                             kind="ExternalOutput").ap()

    with tile.TileContext(nc) as tc:
        with (
            tc.tile_pool(name="const", bufs=1) as constp,
            tc.tile_pool(name="xin", bufs=1) as xinp,
            tc.tile_pool(name="mf", bufs=1) as mfp,
            tc.tile_pool(name="scratch", bufs=1) as scr,
            tc.tile_pool(name="outp", bufs=6) as outp,
        ):
            coef = constp.tile([128, 3 * IN_DIM * N_MF], F32)
            nc.scalar.dma_start(coef[:], coef_ext[:])

            # X in block layout: partition p holds rows p*T .. p*T+T-1
            xt = xinp.tile([128, T * IN_DIM], F32)
            nc.sync.dma_start(
                xt[:].rearrange("p (t d) -> p t d", d=IN_DIM),
                x_ext.rearrange("(p t) d -> p t d", t=T),
            )
            # broadcast X over the m axis: [128, T, D, M]
            xt_b = (xt[:].rearrange("p (t d) -> p t d", d=IN_DIM)
                    .unsqueeze(3).to_broadcast([128, T, IN_DIM, N_MF]))

            def cview(i):  # i-th coef block as [128, T(bcast), D, M]
                return (coef[:, i * IN_DIM * N_MF:(i + 1) * IN_DIM * N_MF]
                        .rearrange("p (d m) -> p d m", m=N_MF)
                        .unsqueeze(1)
                        .to_broadcast([128, T, IN_DIM, N_MF]))

            # mf values, layout (t, d, m), both m computed in one pass
            mfc = mfp.tile([128, T * IN_DIM * N_MF], F32)
            mfc4 = mfc[:].rearrange("p (t d m) -> p t d m", d=IN_DIM, m=N_MF)
            u = scr.tile([128, T * IN_DIM * N_MF], F32, tag="u")
            v = scr.tile([128, T * IN_DIM * N_MF], F32, tag="v")
            u4 = u[:].rearrange("p (t d m) -> p t d m", d=IN_DIM, m=N_MF)
            v4 = v[:].rearrange("p (t d m) -> p t d m", d=IN_DIM, m=N_MF)

            nc.vector.tensor_add(u4, xt_b, cview(0))        # u = x - c
            nc.vector.tensor_mul(v4, u4, cview(2))          # v = -u/rd2
            nc.vector.tensor_mul(u4, u4, cview(1))          # u = u/ld2
            nc.vector.tensor_tensor(u4, u4, v4, OP.min)
            nc.vector.tensor_scalar(mfc4, u4, 1.0, 0.0, OP.add, OP.max)

            # row sum: prod_d (mf0 + mf1), via product tree (6 small ops)
            ps = scr.tile([128, T * IN_DIM], F32, tag="ps")
            ps3 = ps[:].rearrange("p (t d) -> p t d", d=IN_DIM)
            nc.vector.tensor_add(ps3, mfc4[:, :, :, 0], mfc4[:, :, :, 1])
            q5 = scr.tile([128, T * 5], F32, tag="q5")
            q53 = q5[:].rearrange("p (t d) -> p t d", d=5)
            nc.vector.tensor_mul(q53, ps3[:, :, 0:5], ps3[:, :, 5:10])
            r2 = scr.tile([128, T * 2], F32, tag="r2")
            r23 = r2[:].rearrange("p (t d) -> p t d", d=2)
            nc.vector.tensor_mul(r23, q53[:, :, 0:2], q53[:, :, 2:4])
            s1 = scr.tile([128, T], F32, tag="s1")
            nc.vector.tensor_mul(s1[:].unsqueeze(2), r23[:, :, 0:1],
                                 r23[:, :, 1:2])
            nc.vector.tensor_mul(s1[:].unsqueeze(2), s1[:].unsqueeze(2),
                                 q53[:, :, 4:5])
            nc.vector.tensor_scalar_add(s1[:], s1[:], EPS)
            rcp = scr.tile([128, T], F32, tag="rcp")
            nc.vector.reciprocal(rcp[:], s1[:])

            # joint A/B successive doubling, new bit appended HIGH.
            # pair(d) = mf values for dims {d, d+5} -> [128, (t h), m]
            mfp5 = mfc4.rearrange("p t (h dd) m -> p (t h) dd m", h=2)

            def pair(d):
                return mfp5[:, :, d, :]                     # [128, T*2, 2]

            cur = pair(4)                                   # j = bit(d4)
            width = 2
            for k in range(1, 5):
                nxt = scr.tile([128, T * 2 * 2 * width], F32, tag=f"dbl{k}")
                nxt_v = nxt[:].rearrange("p (th i j) -> p th i j",
                                         i=2, j=width)
                nc.vector.tensor_mul(
                    nxt_v,
                    pair(4 - k).unsqueeze(3)
                        .to_broadcast([128, T * 2, 2, width]),
                    cur.unsqueeze(2).to_broadcast([128, T * 2, 2, width]),
                )
                cur = nxt_v.rearrange("p th i j -> p th (i j)")
                width *= 2

            # cur: [128, (t h), 32]; A-half at h=0, B-half at h=1
            hv = cur.rearrange("p (t h) j -> p t h j", h=2)
            A3 = hv[:, :, 0, :]                             # [128, T, 32]
            B3 = hv[:, :, 1, :]

            # fold 1/rowsum into the A half
            nc.vector.tensor_mul(
                A3, A3, rcp[:].unsqueeze(2).to_broadcast([128, T, HALF]))

            out_r = out_ext.rearrange("(p t) r -> p t r", t=T)
            for t in range(T):
                o = outp.tile([128, N_RULE], F32)
                eng = nc.gpsimd if t in GP_GROUPS else nc.vector
                # out = (A/rowsum) (x) B, outer product over (ra, rb)
                eng.tensor_mul(
                    o[:].rearrange("p (a b) -> p a b", b=HALF),
                    A3[:, t, :].unsqueeze(2).to_broadcast([128, HALF, HALF]),
                    B3[:, t, :].unsqueeze(1).to_broadcast([128, HALF, HALF]),
                )
                deng = nc.sync if t % 2 == 0 else nc.scalar
                deng.dma_start(out_r[:, t, :], o[:])

    nc.compile()
    _prog_cache["nc"] = nc
    return nc


def _host_coefs(center, left_dist, right_dist):
    """[128, 60] replicated coefficient tile; blocks (d,m)-interleaved:
    [-center, 1/ld2, -1/rd2]."""
    c = np.asarray(center, np.float32)
    ld2 = np.asarray(left_dist, np.float32) ** 2 + np.float32(EPS)
    rd2 = np.asarray(right_dist, np.float32) ** 2 + np.float32(EPS)
    row = np.concatenate([
        (-c).reshape(-1),
        (1.0 / ld2.astype(np.float64)).astype(np.float32).reshape(-1),
        (-1.0 / rd2.astype(np.float64)).astype(np.float32).reshape(-1),
    ]).astype(np.float32)
    return np.ascontiguousarray(np.broadcast_to(row, (128, row.size)))


def _numpy_reference(X, center, left_dist, right_dist, rule_idx):
    """Safety-net path for non-cartesian rule tables (not the graded case)."""
    X = np.asarray(X, np.float32)
    center = np.asarray(center, np.float32)
    ld2 = np.asarray(left_dist, np.float32) ** 2 + np.float32(EPS)
    rd2 = np.asarray(right_dist, np.float32) ** 2 + np.float32(EPS)
    left = X[:, :, None] / ld2 + 1.0 - center / ld2
    right = -X[:, :, None] / rd2 + 1.0 + center / rd2
    mf = np.maximum(0.0, np.minimum(left, right)).astype(np.float32)
    frs = np.ones((X.shape[0], rule_idx.shape[0]), np.float32)
    for d in range(IN_DIM):
        frs = frs * mf[:, d, rule_idx[:, d]]
    return frs / (frs.sum(axis=1, keepdims=True) + np.float32(EPS))


def kernel(X, center, left_dist, right_dist, rule_idx):
    X = np.ascontiguousarray(np.asarray(X, np.float32))
    rule_idx = np.asarray(rule_idx, np.int32)
    assert X.shape == (BATCH, IN_DIM)

    # fast path requires the standard cartesian-product rule table
    # (itertools.product order: dim 0 is the most significant bit)
    weights = (2 ** np.arange(IN_DIM - 1, -1, -1)).astype(np.int64)
    codes = rule_idx.astype(np.int64) @ weights
    if (rule_idx.shape != (N_RULE, IN_DIM)
            or rule_idx.min() < 0 or rule_idx.max() >= N_MF
            or not np.array_equal(codes, np.arange(N_RULE))):
        return _numpy_reference(X, center, left_dist, right_dist, rule_idx)

    from concourse import bass_utils

    nc = _build_program()
    coef = _host_coefs(center, left_dist, right_dist)
    in_maps = [
        {"X": np.ascontiguousarray(X[c * SHARD:(c + 1) * SHARD]), "coef": coef}
        for c in range(N_CORES)
    ]
    res = bass_utils.run_bass_kernel_spmd(
        nc, in_maps, core_ids=list(range(N_CORES)))
    return np.concatenate([res.results[c]["out"] for c in range(N_CORES)],
                          axis=0)
